# revision 35
# baseline (speedup 1.0000x reference)
"""Trainium2 Bass kernel for causal GQA self-attention (B=2,S=2048,D=1024,H=16,HKV=4,HD=64).

Sharding: 8 cores = DP(2 over batch) x TP(4 over GQA groups).
I/O-lean layout:
  - x arrives seq-sharded: core (dp,tp) receives xT[:, tp*512:(tp+1)*512] of batch dp
    (1MB bf16); an on-device AllGather over the DP group rebuilds the full xT.
  - each core computes its GQA group's attention partial y_group @ Wo[:, group].T,
    then an on-device ReduceScatter sums the 4 TP partials and hands core tp
    rows [tp*512:(tp+1)*512]; the core returns that chunk in bf16 (1MB).
Host just concatenates chunks - no reduction on host.
"""

import sys
from contextlib import ExitStack

sys.path.insert(0, "/opt/trn_rl_repo")

import numpy as np
import ml_dtypes

import concourse.bass as bass
import concourse.bacc as bacc
import concourse.tile as tile
import concourse.mybir as mybir
from concourse.bass_utils import run_bass_kernel_spmd

BF16 = mybir.dt.bfloat16
F32 = mybir.dt.float32
AF = mybir.ActivationFunctionType
BF16NP = ml_dtypes.bfloat16

D, H, HKV, HD, B, S = 1024, 16, 4, 64, 2, 2048
HG = 4              # q heads per core
E = HG * HD         # 256 local q-proj dim
ROPE_BASE = 10000.0
EPS = float(np.finfo(np.float32).eps)

NK = D // 128       # 8 contraction tiles for qkv projections
SQB = 256           # sq block size in attention
NB = S // SQB       # 8 blocks
NJ = S // 128       # 16 sk tiles
SC = S // 4         # 512 seq chunk per core
GROUPS = [[0, 1, 2, 3], [4, 5, 6, 7]]


def _consts():
    """Constant tensors baked into the NEFF (same for every core)."""
    i = np.arange(32, dtype=np.float64)
    inv_freq = 1.0 / (ROPE_BASE ** (2.0 * i / HD))
    pos = np.arange(S, dtype=np.float64)
    fr = pos[:, None] * inv_freq[None, :]          # [S, 32]
    cosT = np.cos(fr).T.astype(np.float32)          # [32, S]
    sinT = np.sin(fr).T.astype(np.float32)
    cos4 = np.tile(cosT, (4, 1)).astype(BF16NP)     # [128, S]
    sin4 = np.tile(sinT, (4, 1)).astype(BF16NP)
    nsin4 = (-np.tile(sinT, (4, 1))).astype(BF16NP)

    # causal masks for diagonal sk-tiles: pattern p in {0,1}
    # valid iff c >= 128*p + r   (r: sk row 0..127, c: sq col 0..255)
    r = np.arange(128)[:, None]
    c = np.arange(SQB)[None, :]
    masks = []
    for p in range(2):
        m = (c >= 128 * p + r).astype(BF16NP)       # [128, 256]
        masks.append(np.tile(m, (1, HG)))            # [128, 1024] (4 head blocks)

    bsel4 = np.zeros((4, 128), dtype=BF16NP)        # broadcast f[h] -> rows 32h..32h+32
    for h in range(4):
        bsel4[h, 32 * h:32 * h + 32] = 1.0
    sel4 = bsel4.T.copy()                            # [128, 4] sumsq selector
    ones64col = np.ones((64, 1), dtype=BF16NP)
    id128 = np.eye(128, dtype=BF16NP)
    return cos4, sin4, nsin4, masks, bsel4, sel4, ones64col, id128


def _build():
    nc = bacc.Bacc("TRN2", debug=False)

    xs_d = nc.dram_tensor("xs", [D, SC], BF16, kind="ExternalInput")
    # packed half of this TP shard's weights: [wq k0-3 | wkv k0-3 | wo kk] halves
    WHW = (NK * E + NK * 128 + 2 * D) // 2          # 2560
    wh_d = nc.dram_tensor("wh", [128, WHW], BF16, kind="ExternalInput")
    qlnb_d = nc.dram_tensor("qlnb", [4, 1], F32, kind="ExternalInput")
    out_d = nc.dram_tensor("out", [SC, D], BF16, kind="ExternalOutput")
    import os
    KDEBUG = int(os.environ.get("KDEBUG", "0"))
    dbg = {}
    if KDEBUG:
        for nm, shp in [("d_qsb0", [128, S]), ("d_qsb1", [128, S]),
                        ("d_kvsb", [128, S]), ("d_fq", [4, S]), ("d_fk", [1, S]),
                        ("d_qr0", [128, S]), ("d_qr1", [128, S]),
                        ("d_kdup", [128, S]), ("d_qAB", [128, 2 * S]),
                        ("d_vsb", [128, NJ * 65]),
                        ("d_pt0", [128, HG * SQB]), ("d_pt1", [128, HG * SQB]),
                        ("d_yn0", [128, S]), ("d_yn1", [128, S])]:
            dbg[nm] = nc.dram_tensor(nm, shp, BF16, kind="ExternalOutput")

    cos4, sin4, nsin4, masks, bsel4, sel4, ones64col, id128 = _consts()
    cos4_d = nc.inline_tensor(cos4, "cos4")
    sin4_d = nc.inline_tensor(sin4, "sin4")
    nsin4_d = nc.inline_tensor(nsin4, "nsin4")
    mask_d = [nc.inline_tensor(masks[p], f"mask{p}") for p in range(2)]
    bsel4_d = nc.inline_tensor(bsel4, "bsel4")
    sel4_d = nc.inline_tensor(sel4, "sel4")
    ones64col_d = nc.inline_tensor(ones64col, "ones64col")
    id128_d = nc.inline_tensor(id128, "id128")

    with tile.TileContext(nc) as tc, ExitStack() as ctx:
        sp = ctx.enter_context(tc.tile_pool(name="static", bufs=1))
        dram = ctx.enter_context(tc.tile_pool(name="dram", bufs=1, space="DRAM"))

        def stile(shape, dt, tag):
            return sp.tile(shape, dt, name=tag, tag=tag)

        # ---- DRAM internals for collectives ----
        xgi = [dram.tile([512, SC], BF16, name=f"xgi{h}", tag=f"xgi{h}")
               for h in range(2)]
        xgo = [dram.tile([4, 512, SC], BF16, name=f"xgo{h}", tag=f"xgo{h}")
               for h in range(2)]
        whb = dram.tile([128, WHW], BF16, name="whb", tag="whb")
        whg = dram.tile([2, 128, WHW], BF16, name="whg", tag="whg")
        yb = [dram.tile([S, 512], BF16, name=f"yb{n}", tag=f"yb{n}")
              for n in range(2)]
        rso = [dram.tile([SC, 512], BF16, name=f"rso{n}", tag=f"rso{n}")
               for n in range(2)]

        # ---- static SBUF tensors ----
        xc = [stile([128, S], BF16, f"xc{k}") for k in range(NK)]
        wq = stile([128, NK * E], BF16, "wq")
        wkv = stile([128, NK * 128], BF16, "wkv")
        wo = stile([128, 2 * D], BF16, "wo")
        cos4_s = stile([128, S], BF16, "cos4")
        sin4_s = stile([128, S], BF16, "sin4")
        nsin4_s = stile([128, S], BF16, "nsin4")
        mask_s = [stile([128, HG * SQB], BF16, f"mask{p}") for p in range(2)]
        bsel4_s = stile([4, 128], BF16, "bsel4")
        sel4_s = stile([128, 4], BF16, "sel4")
        ones64col_s = stile([64, 1], BF16, "ones64col")
        id128_s = stile([128, 128], BF16, "id128")
        qlnb_s = stile([4, 1], F32, "qlnb")
        epsb = stile([128, 1], F32, "epsb")
        zb = stile([128, 1], F32, "zb")

        qsb = [stile([128, S], BF16, f"qsb{m}") for m in range(2)]   # T/B packed
        kvsb = stile([128, S], BF16, "kvsb")                          # k(0:64) | v(64:128)
        sqq = [stile([128, S], BF16, f"sqq{m}") for m in range(2)]
        sqkv = stile([64, S], BF16, "sqkv")
        lnq = stile([4, S], F32, "lnq")
        lnk = stile([1, S], F32, "lnk")
        fq = stile([4, S], BF16, "fq")
        fk = stile([1, S], BF16, "fk")
        fbcq = stile([128, S], BF16, "fbcq")
        fbck = stile([32, S], BF16, "fbck")
        qr = [stile([128, S], BF16, f"qr{m}") for m in range(2)]      # rotated T/B
        kr = [stile([32, S], BF16, f"kr{m}") for m in range(2)]
        kb0 = stile([32, S], BF16, "kb0")
        qAB = stile([128, 2, S], BF16, "qAB")  # rows 0:64 heads 0|1, 64:128 heads 2|3
        kdup = stile([128, S], BF16, "kdup")
        onesq = stile([128, 64], BF16, "onesq")
        vsb = stile([128, NJ, 65], BF16, "vsb")                       # [v | ones]
        yn = [stile([128, S], BF16, f"yn{m}") for m in range(2)]      # normalized y^T
        ynodd = [stile([64, S], BF16, f"ynodd{m}") for m in range(2)]

        # ---- AllGather x across the DP group (two D-halves, so the first
        # projections overlap the second half's comm) and the weight halves
        # across DP pairs (each weight shard is sent to only one dp) ----
        nc.sync.dma_start(whb[:], wh_d[:])
        nc.sync.dma_start(xgi[0][:], xs_d[0:512, :])
        nc.sync.dma_start(xgi[1][:], xs_d[512:1024, :])
        nc.gpsimd.collective_compute(
            "AllGather", mybir.AluOpType.bypass,
            replica_groups=[[0, 4], [1, 5], [2, 6], [3, 7]],
            ins=[whb.opt()], outs=[whg.opt()])
        nc.gpsimd.collective_compute(
            "AllGather", mybir.AluOpType.bypass, replica_groups=GROUPS,
            ins=[xgi[0].opt()], outs=[xgo[0].opt()])
        nc.gpsimd.collective_compute(
            "AllGather", mybir.AluOpType.bypass, replica_groups=GROUPS,
            ins=[xgi[1].opt()], outs=[xgo[1].opt()])
        # unpack gathered weight halves: half d = k-tiles 4d..4d+3 (wq, wkv)
        # and kk=d block (wo)
        for d in range(2):
            nc.sync.dma_start(wq[:, 1024 * d:1024 * (d + 1)],
                              whg[d, :, 0:1024])
            nc.sync.dma_start(wkv[:, 512 * d:512 * (d + 1)],
                              whg[d, :, 1024:1536])
            nc.sync.dma_start(wo[:, 1024 * d:1024 * (d + 1)],
                              whg[d, :, 1536:2560])
        nc.sync.dma_start(cos4_s[:], cos4_d[:])
        nc.sync.dma_start(sin4_s[:], sin4_d[:])
        nc.sync.dma_start(nsin4_s[:], nsin4_d[:])
        for p in range(2):
            nc.sync.dma_start(mask_s[p][:], mask_d[p][:])
        nc.sync.dma_start(bsel4_s[:], bsel4_d[:])
        nc.sync.dma_start(sel4_s[:], sel4_d[:])
        nc.sync.dma_start(ones64col_s[:], ones64col_d[:])
        nc.sync.dma_start(id128_s[:], id128_d[:])
        nc.sync.dma_start(qlnb_s[:], qlnb_d[:])
        nc.vector.memset(vsb[:], 1.0)  # ones column at [:, j, 64]; 0:64 overwritten below
        nc.vector.memset(epsb[:], EPS)
        nc.vector.memset(zb[:], 0.0)
        nc.vector.memset(onesq[:], 1.0)

        # gathered x -> SBUF tiles  xc[k][p, 512c:512c+512] = xT[128k+p, chunk c]
        for k in range(NK):
            h, kr_ = divmod(k, 4)
            for c in range(4):
                nc.sync.dma_start(xc[k][:, 512 * c:512 * (c + 1)],
                                  xgo[h][c, 128 * kr_:128 * (kr_ + 1), :])

        # ======== phase 1: projections + rms factors + rope ========
        with (
            tc.tile_pool(name="pp", bufs=1, space=bass.MemorySpace.PSUM) as pp,
        ):
            # Q projection -> qsb (permuted: tileT = tops of 4 heads, tileB = bottoms)
            for m in range(2):
                pq = pp.tile([128, S], F32, name="pq", tag="pq")
                for k in range(NK):
                    for n in range(4):
                        nc.tensor.matmul(
                            pq[:, 512 * n:512 * (n + 1)],
                            wq[:, 256 * k + 128 * m:256 * k + 128 * (m + 1)],
                            xc[k][:, 512 * n:512 * (n + 1)],
                            start=(k == 0), stop=(k == NK - 1))
                nc.scalar.copy(qsb[m][:], pq[:])
                nc.vector.tensor_mul(sqq[m][:], qsb[m][:], qsb[m][:])
            # KV projection
            pkv = pp.tile([128, S], F32, name="pq", tag="pq")
            for k in range(NK):
                for n in range(4):
                    nc.tensor.matmul(
                        pkv[:, 512 * n:512 * (n + 1)],
                        wkv[:, 128 * k:128 * (k + 1)],
                        xc[k][:, 512 * n:512 * (n + 1)],
                        start=(k == 0), stop=(k == NK - 1))
            nc.scalar.copy(kvsb[:], pkv[:])
            nc.vector.tensor_mul(sqkv[:], kvsb[0:64, :], kvsb[0:64, :])
            # v transpose: [64,128] slices -> [128,64] columns of vsb
            for st in range(NJ):
                ptr = pp.tile([128, 64], BF16, name="ptr", tag="ptr", bufs=2)
                nc.tensor.transpose(
                    ptr[:], kvsb[64:128, 128 * st:128 * (st + 1)],
                    id128_s[64:128, 64:128])
                nc.vector.tensor_copy(vsb[:, st, 0:64], ptr[:])

            # rms factors: f = exp(-0.5*ln(ssq/HD + eps) + ln(gain/8))
            psq = pp.tile([4, S], F32, name="psq", tag="pq")
            for n in range(4):
                sl = slice(512 * n, 512 * (n + 1))
                nc.tensor.matmul(psq[0:4, sl], sel4_s[:], sqq[0][:, sl],
                                 start=True, stop=False)
                nc.tensor.matmul(psq[0:4, sl], sel4_s[:], sqq[1][:, sl],
                                 start=False, stop=True)
            nc.scalar.activation(lnq[:], psq[:], AF.Ln, scale=1.0 / HD,
                                 bias=epsb[0:4, :])
            psk = pp.tile([1, S], F32, name="psk", tag="pq")
            for n in range(4):
                sl = slice(512 * n, 512 * (n + 1))
                nc.tensor.matmul(psk[0:1, sl], ones64col_s[:], sqkv[:, sl],
                                 start=True, stop=True)
            nc.scalar.activation(lnk[:], psk[:], AF.Ln, scale=1.0 / HD,
                                 bias=epsb[0:1, :])
            nc.scalar.activation(fq[:], lnq[:], AF.Exp, scale=-0.5, bias=qlnb_s[:, :])
            nc.scalar.activation(fk[:], lnk[:], AF.Exp, scale=-0.5, bias=zb[0:1, :])
            # broadcast factors along hd rows via PE
            pbq = pp.tile([128, S], F32, name="pbq", tag="pq")
            for n in range(4):
                sl = slice(512 * n, 512 * (n + 1))
                nc.tensor.matmul(pbq[:, sl], bsel4_s[:], fq[:, sl],
                                 start=True, stop=True)
            nc.scalar.copy(fbcq[:], pbq[:])
            pbk = pp.tile([32, S], F32, name="pbk", tag="pq")
            for n in range(4):
                sl = slice(512 * n, 512 * (n + 1))
                nc.tensor.matmul(pbk[0:32, sl], onesq[0:1, 0:32], fk[:, sl],
                                 start=True, stop=True)
            nc.scalar.copy(fbck[:], pbk[:])

            # k bottom half shifted to partition base 0 (DVE ops need aligned bases)
            nc.sync.dma_start(kb0[:], kvsb[32:64, :])

            # rope + scale (DVE, bf16), in two column halves so the first
            # attention blocks start before the second half finishes
            with tc.tile_pool(name="rt", bufs=2) as rt:
                for hf in range(2):
                    sl = slice(1024 * hf, 1024 * (hf + 1))
                    t1 = rt.tile([128, 1024], BF16, name="t1", tag="t1")
                    t2 = rt.tile([128, 1024], BF16, name="t2", tag="t2")
                    nc.vector.tensor_mul(t1[:], qsb[0][:, sl], cos4_s[:, sl])
                    nc.vector.tensor_mul(t2[:], qsb[1][:, sl], sin4_s[:, sl])
                    nc.vector.tensor_add(t1[:], t1[:], t2[:])
                    nc.vector.tensor_mul(qr[0][:, sl], t1[:], fbcq[:, sl])
                    u1 = rt.tile([128, 1024], BF16, name="t1", tag="t1")
                    u2 = rt.tile([128, 1024], BF16, name="t2", tag="t2")
                    nc.vector.tensor_mul(u1[:], qsb[0][:, sl], nsin4_s[:, sl])
                    nc.vector.tensor_mul(u2[:], qsb[1][:, sl], cos4_s[:, sl])
                    nc.vector.tensor_add(u1[:], u1[:], u2[:])
                    nc.vector.tensor_mul(qr[1][:, sl], u1[:], fbcq[:, sl])
                    k1 = rt.tile([32, 1024], BF16, name="k1", tag="k1")
                    k2 = rt.tile([32, 1024], BF16, name="k2", tag="k2")
                    nc.vector.tensor_mul(k1[:], kvsb[0:32, sl], cos4_s[0:32, sl])
                    nc.vector.tensor_mul(k2[:], kb0[:, sl], sin4_s[0:32, sl])
                    nc.vector.tensor_add(k1[:], k1[:], k2[:])
                    nc.vector.tensor_mul(kr[0][:, sl], k1[:], fbck[:, sl])
                    k3 = rt.tile([32, 1024], BF16, name="k1", tag="k1")
                    k4 = rt.tile([32, 1024], BF16, name="k2", tag="k2")
                    nc.vector.tensor_mul(k3[:], kvsb[0:32, sl], nsin4_s[0:32, sl])
                    nc.vector.tensor_mul(k4[:], kb0[:, sl], cos4_s[0:32, sl])
                    nc.vector.tensor_add(k3[:], k3[:], k4[:])
                    nc.vector.tensor_mul(kr[1][:, sl], k3[:], fbck[:, sl])
                    # reassemble per-head-pair layout qAB[dim, head%2, s] for
                    # this half: head h dims = [qr[0] (rot top); qr[1] (bot)]
                    nc.sync.dma_start(kdup[0:32, sl], kr[0][:, sl])
                    nc.sync.dma_start(kdup[32:64, sl], kr[1][:, sl])
                    nc.sync.dma_start(kdup[64:96, sl], kr[0][:, sl])
                    nc.sync.dma_start(kdup[96:128, sl], kr[1][:, sl])
                    for h in range(4):
                        pb_ = 64 * (h // 2)   # pair A rows 0:64, pair B 64:128
                        fi = h % 2            # free index within the pair
                        nc.sync.dma_start(qAB[pb_:pb_ + 32, fi, sl],
                                          qr[0][32 * h:32 * h + 32, sl])
                        nc.sync.dma_start(qAB[pb_ + 32:pb_ + 64, fi, sl],
                                          qr[1][32 * h:32 * h + 32, sl])

        # ======== phase 2 + fused output projection ========
        with (
            tc.tile_pool(name="ps", bufs=2, space=bass.MemorySpace.PSUM) as ps,
            tc.tile_pool(name="py", bufs=2, space=bass.MemorySpace.PSUM) as py,
            tc.tile_pool(name="po", bufs=2, space=bass.MemorySpace.PSUM) as po,
            tc.tile_pool(name="pa", bufs=3) as pa,
            tc.tile_pool(name="ob", bufs=3) as ob,
        ):
            for b in range(NB):
                sq = slice(SQB * b, SQB * (b + 1))
                jmax = 2 * b + 1
                # yt[pair] accumulates [v|ones]^T @ p : rows 0:64 y, row 64 denom
                yt = [py.tile([65, 512], F32, name="yt", tag="yt") for _ in range(2)]
                for j in range(jmax + 1):
                    stile_ = ps.tile([128, HG * SQB], F32, name="st", tag="st")
                    for pr in range(2):
                        base = 64 * pr
                        nc.tensor.matmul(
                            stile_[:, 512 * pr:512 * (pr + 1)],
                            kdup[base:base + 64, 128 * j:128 * (j + 1)],
                            qAB[base:base + 64, :, sq],
                            start=True, stop=True, skip_group_check=True)
                    pt = pa.tile([128, HG * SQB], BF16, name="pt", tag="pt")
                    nc.scalar.activation(pt[:], stile_[:], AF.Exp, bias=zb[:, :])
                    if j >= 2 * b:
                        nc.vector.tensor_mul(pt[:], pt[:], mask_s[j - 2 * b][:])
                    if KDEBUG and b == 0:
                        nc.sync.dma_start(dbg[f"d_pt{j}"][:], pt[:])
                    for pr in range(2):
                        nc.tensor.matmul(
                            yt[pr][:], vsb[:, j, :], pt[:, 512 * pr:512 * (pr + 1)],
                            start=(j == 0), stop=(j == jmax))
                # normalize: y / denom -> yn rows (even head) + ynodd staging (odd)
                for pr in range(2):
                    dcb = pa.tile([1, 512], BF16, name="dcb", tag="dcb")
                    nc.vector.tensor_copy(dcb[:], yt[pr][64:65, :])
                    prb = ps.tile([64, 512], F32, name="prb", tag="st")
                    nc.tensor.matmul(prb[:], onesq[0:1, 0:64], dcb[:],
                                     start=True, stop=True)
                    rbs = pa.tile([64, 512], F32, name="rbs", tag="rbs")
                    nc.vector.reciprocal_approx_fast(rbs[:], prb[:])
                    nc.vector.tensor_mul(yn[pr][0:64, sq], yt[pr][0:64, 0:256],
                                         rbs[:, 0:256])
                    nc.vector.tensor_mul(ynodd[pr][:, sq], yt[pr][0:64, 256:512],
                                         rbs[:, 256:512])
                    nc.sync.dma_start(yn[pr][64:128, sq], ynodd[pr][:, sq])
                # fused output projection for this block's two row-tiles
                for st in (2 * b, 2 * b + 1):
                    ssl = slice(128 * st, 128 * (st + 1))
                    for n in range(2):
                        pot = po.tile([128, 512], F32, name="po", tag="po")
                        for kk in range(2):
                            nc.tensor.matmul(
                                pot[:], yn[kk][:, ssl],
                                wo[:, 1024 * kk + 512 * n:1024 * kk + 512 * (n + 1)],
                                start=(kk == 0), stop=(kk == 1))
                        ot = ob.tile([128, 512], BF16, name="ot", tag="ot")
                        nc.vector.tensor_copy(ot[:], pot[:])
                        nc.sync.dma_start(yb[n][ssl, :], ot[:])
        if KDEBUG:
            nc.sync.dma_start(dbg["d_qsb0"][:], qsb[0][:])
            nc.sync.dma_start(dbg["d_qsb1"][:], qsb[1][:])
            nc.sync.dma_start(dbg["d_kvsb"][:], kvsb[:])
            nc.sync.dma_start(dbg["d_fq"][:], fq[:])
            nc.sync.dma_start(dbg["d_fk"][:], fk[:])
            nc.sync.dma_start(dbg["d_qr0"][:], qr[0][:])
            nc.sync.dma_start(dbg["d_qr1"][:], qr[1][:])
            nc.sync.dma_start(dbg["d_kdup"][:], kdup[:])
            nc.sync.dma_start(dbg["d_qAB"][:, 0:S], qAB[:, 0, :])
            nc.sync.dma_start(dbg["d_qAB"][:, S:2 * S], qAB[:, 1, :])
            for jj in range(NJ):
                nc.sync.dma_start(dbg["d_vsb"][:, 65 * jj:65 * (jj + 1)],
                                  vsb[:, jj, :])
            nc.sync.dma_start(dbg["d_yn0"][:], yn[0][:])
            nc.sync.dma_start(dbg["d_yn1"][:], yn[1][:])

        # ======== ReduceScatter the partials, write the output chunk ========
        for n in range(2):
            nc.gpsimd.collective_compute(
                "ReduceScatter", mybir.AluOpType.add, replica_groups=GROUPS,
                ins=[yb[n].opt()], outs=[rso[n].opt()])
        for n in range(2):
            nc.sync.dma_start(out_d[:, 512 * n:512 * (n + 1)], rso[n][:])

    nc.finalize()
    return nc


_NC = None


def _get_nc():
    global _NC
    if _NC is None:
        _NC = _build()
    return _NC


def _perm():
    tops = [h * 64 + i for h in range(HG) for i in range(32)]
    bots = [h * 64 + 32 + i for h in range(HG) for i in range(32)]
    return tops + bots


def _build_in_maps(x, Wq, Wk, Wv, Wo, q_gain):
    perm = _perm()
    in_maps = []
    for c in range(8):
        dp, tp = divmod(c, 4)
        xs = np.ascontiguousarray(
            x[dp].T[:, tp * SC:(tp + 1) * SC]).astype(BF16NP)     # [D, 512]
        wq_sel = Wq[tp * E:(tp + 1) * E].T[:, perm]               # [D, 256] permuted
        wq_t = np.ascontiguousarray(
            wq_sel.reshape(NK, 128, E).transpose(1, 0, 2).reshape(128, NK * E)
        ).astype(BF16NP)
        wk_sel = Wk[tp * HD:(tp + 1) * HD].T                      # [D, 64]
        wv_sel = Wv[tp * HD:(tp + 1) * HD].T
        wkv_sel = np.concatenate([wk_sel, wv_sel], axis=1)        # [D, 128]
        wkv_t = np.ascontiguousarray(
            wkv_sel.reshape(NK, 128, 128).transpose(1, 0, 2).reshape(128, NK * 128)
        ).astype(BF16NP)
        wo_sel = Wo[:, tp * E:(tp + 1) * E].T                     # [256, D]
        wo_t = np.ascontiguousarray(
            wo_sel.reshape(2, 128, D).transpose(1, 0, 2).reshape(128, 2 * D)
        ).astype(BF16NP)
        # each core ships only half its weight shard; the DP-pair AllGather
        # reassembles the full shard on device
        wh = np.concatenate([wq_t[:, 1024 * dp:1024 * (dp + 1)],
                             wkv_t[:, 512 * dp:512 * (dp + 1)],
                             wo_t[:, 1024 * dp:1024 * (dp + 1)]], axis=1)
        g = q_gain[tp * HG:(tp + 1) * HG].astype(np.float64)
        qlnb = np.log(np.maximum(g, 1e-30) / 8.0).astype(np.float32).reshape(4, 1)
        in_maps.append({
            "xs": xs, "wh": np.ascontiguousarray(wh), "qlnb": qlnb,
        })
    return in_maps


def kernel(x, Wq, Wk, Wv, Wo, q_gain):
    x = np.asarray(x, dtype=np.float32)
    Wq = np.asarray(Wq, dtype=np.float32)
    Wk = np.asarray(Wk, dtype=np.float32)
    Wv = np.asarray(Wv, dtype=np.float32)
    Wo = np.asarray(Wo, dtype=np.float32)
    q_gain = np.asarray(q_gain, dtype=np.float32)

    in_maps = _build_in_maps(x, Wq, Wk, Wv, Wo, q_gain)
    nc = _get_nc()
    res = run_bass_kernel_spmd(nc, in_maps, core_ids=list(range(8)))
    out = np.zeros((B, S, D), dtype=np.float32)
    for c in range(8):
        dp, tp = divmod(c, 4)
        out[dp, tp * SC:(tp + 1) * SC, :] = res.results[c]["out"].astype(np.float32)
    return out


# revision 41
# speedup vs baseline: 1.0383x; 1.0383x over previous
"""Trainium2 Bass kernel for causal GQA self-attention (B=2,S=2048,D=1024,H=16,HKV=4,HD=64).

Sharding: 8 cores = DP(2 over batch) x TP(4 over GQA groups).
I/O-lean layout:
  - x arrives seq-sharded: core (dp,tp) receives xT[:, tp*512:(tp+1)*512] of batch dp
    (1MB bf16); an on-device AllGather over the DP group rebuilds the full xT.
  - each core computes its GQA group's attention partial y_group @ Wo[:, group].T,
    then an on-device ReduceScatter sums the 4 TP partials and hands core tp
    rows [tp*512:(tp+1)*512]; the core returns that chunk in bf16 (1MB).
Host just concatenates chunks - no reduction on host.
"""

import sys
from contextlib import ExitStack

sys.path.insert(0, "/opt/trn_rl_repo")

import numpy as np
import ml_dtypes

import concourse.bass as bass
import concourse.bacc as bacc
import concourse.tile as tile
import concourse.mybir as mybir
from concourse.bass_utils import run_bass_kernel_spmd

BF16 = mybir.dt.bfloat16
F32 = mybir.dt.float32
AF = mybir.ActivationFunctionType
BF16NP = ml_dtypes.bfloat16

D, H, HKV, HD, B, S = 1024, 16, 4, 64, 2, 2048
HG = 4              # q heads per core
E = HG * HD         # 256 local q-proj dim
ROPE_BASE = 10000.0
EPS = float(np.finfo(np.float32).eps)

NK = D // 128       # 8 contraction tiles for qkv projections
SQB = 256           # sq block size in attention
NB = S // SQB       # 8 blocks
NJ = S // 128       # 16 sk tiles
SC = S // 4         # 512 seq chunk per core
GROUPS = [[0, 1, 2, 3], [4, 5, 6, 7]]


def _consts():
    """Constant tensors baked into the NEFF (same for every core)."""
    i = np.arange(32, dtype=np.float64)
    inv_freq = 1.0 / (ROPE_BASE ** (2.0 * i / HD))
    pos = np.arange(S, dtype=np.float64)
    fr = pos[:, None] * inv_freq[None, :]          # [S, 32]
    cosT = np.cos(fr).T.astype(np.float32)          # [32, S]
    sinT = np.sin(fr).T.astype(np.float32)
    cos4 = np.tile(cosT, (4, 1)).astype(BF16NP)     # [128, S]
    sin4 = np.tile(sinT, (4, 1)).astype(BF16NP)
    nsin4 = (-np.tile(sinT, (4, 1))).astype(BF16NP)

    # causal masks for diagonal sk-tiles: pattern p in {0,1}
    # valid iff c >= 128*p + r   (r: sk row 0..127, c: sq col 0..255)
    r = np.arange(128)[:, None]
    c = np.arange(SQB)[None, :]
    masks = []
    for p in range(2):
        m = (c >= 128 * p + r).astype(BF16NP)       # [128, 256]
        masks.append(np.tile(m, (1, HG)))            # [128, 1024] (4 head blocks)

    bsel4 = np.zeros((4, 128), dtype=BF16NP)        # broadcast f[h] -> rows 32h..32h+32
    for h in range(4):
        bsel4[h, 32 * h:32 * h + 32] = 1.0
    sel4 = bsel4.T.copy()                            # [128, 4] sumsq selector
    ones64col = np.ones((64, 1), dtype=BF16NP)
    id128 = np.eye(128, dtype=BF16NP)
    return cos4, sin4, nsin4, masks, bsel4, sel4, ones64col, id128


def _build():
    nc = bacc.Bacc("TRN2", debug=False)

    xs_d = nc.dram_tensor("xs", [D, SC], BF16, kind="ExternalInput")
    # packed half of this TP shard's weights: [wq k0-3 | wkv k0-3 | wo kk] halves
    WHW = (NK * E + NK * 128 + 2 * D) // 2          # 2560
    wh_d = nc.dram_tensor("wh", [128, WHW], BF16, kind="ExternalInput")
    qlnb_d = nc.dram_tensor("qlnb", [4, 1], F32, kind="ExternalInput")
    out_d = nc.dram_tensor("out", [SC, D], BF16, kind="ExternalOutput")
    import os
    KDEBUG = int(os.environ.get("KDEBUG", "0"))
    dbg = {}
    if KDEBUG:
        for nm, shp in [("d_qsb0", [128, S]), ("d_qsb1", [128, S]),
                        ("d_kvsb", [128, S]), ("d_fq", [4, S]), ("d_fk", [1, S]),
                        ("d_qr0", [128, S]), ("d_qr1", [128, S]),
                        ("d_kdup", [128, S]), ("d_qAB", [128, 2 * S]),
                        ("d_vsb", [128, NJ * 65]),
                        ("d_pt0", [128, HG * SQB]), ("d_pt1", [128, HG * SQB]),
                        ("d_yn0", [128, S]), ("d_yn1", [128, S])]:
            dbg[nm] = nc.dram_tensor(nm, shp, BF16, kind="ExternalOutput")

    cos4, sin4, nsin4, masks, bsel4, sel4, ones64col, id128 = _consts()
    cos4_d = nc.inline_tensor(cos4, "cos4")
    sin4_d = nc.inline_tensor(sin4, "sin4")
    nsin4_d = nc.inline_tensor(nsin4, "nsin4")
    mask_d = [nc.inline_tensor(masks[p], f"mask{p}") for p in range(2)]
    bsel4_d = nc.inline_tensor(bsel4, "bsel4")
    sel4_d = nc.inline_tensor(sel4, "sel4")
    ones64col_d = nc.inline_tensor(ones64col, "ones64col")
    id128_d = nc.inline_tensor(id128, "id128")

    with tile.TileContext(nc) as tc, ExitStack() as ctx:
        sp = ctx.enter_context(tc.tile_pool(name="static", bufs=1))
        dram = ctx.enter_context(tc.tile_pool(name="dram", bufs=1, space="DRAM"))

        def stile(shape, dt, tag):
            return sp.tile(shape, dt, name=tag, tag=tag)

        # ---- DRAM internals for collectives ----
        xgi = dram.tile([D, SC], BF16, name="xgi", tag="xgi")
        xgo = dram.tile([4, D, SC], BF16, name="xgo", tag="xgo")
        whb = dram.tile([128, WHW], BF16, name="whb", tag="whb")
        whg = dram.tile([2, 128, WHW], BF16, name="whg", tag="whg")
        yb = dram.tile([S, D], BF16, name="yb", tag="yb")
        rso = dram.tile([SC, D], BF16, name="rso", tag="rso")

        # ---- static SBUF tensors ----
        xc = [stile([128, S], BF16, f"xc{k}") for k in range(NK)]
        wq = stile([128, NK * E], BF16, "wq")
        wkv = stile([128, NK * 128], BF16, "wkv")
        wo = stile([128, 2 * D], BF16, "wo")
        cos4_s = stile([128, S], BF16, "cos4")
        sin4_s = stile([128, S], BF16, "sin4")
        nsin4_s = stile([128, S], BF16, "nsin4")
        mask_s = [stile([128, HG * SQB], BF16, f"mask{p}") for p in range(2)]
        bsel4_s = stile([4, 128], BF16, "bsel4")
        sel4_s = stile([128, 4], BF16, "sel4")
        ones64col_s = stile([64, 1], BF16, "ones64col")
        id128_s = stile([128, 128], BF16, "id128")
        qlnb_s = stile([4, 1], F32, "qlnb")
        epsb = stile([128, 1], F32, "epsb")
        zb = stile([128, 1], F32, "zb")

        qsb = [stile([128, S], BF16, f"qsb{m}") for m in range(2)]   # T/B packed
        kvsb = stile([128, S], BF16, "kvsb")                          # k(0:64) | v(64:128)
        sqq = [stile([128, S], BF16, f"sqq{m}") for m in range(2)]
        sqkv = stile([64, S], BF16, "sqkv")
        lnq = stile([4, S], F32, "lnq")
        lnk = stile([1, S], F32, "lnk")
        fq = stile([4, S], BF16, "fq")
        fk = stile([1, S], BF16, "fk")
        fbcq = stile([128, S], BF16, "fbcq")
        fbck = stile([32, S], BF16, "fbck")
        qr = [stile([128, S], BF16, f"qr{m}") for m in range(2)]      # rotated T/B
        kr = [stile([32, S], BF16, f"kr{m}") for m in range(2)]
        kb0 = stile([32, S], BF16, "kb0")
        qAB = stile([128, 2, S], BF16, "qAB")  # rows 0:64 heads 0|1, 64:128 heads 2|3
        kdup = stile([128, S], BF16, "kdup")
        onesq = stile([128, 64], BF16, "onesq")
        vsb = stile([128, NJ, 65], BF16, "vsb")                       # [v | ones]
        yn = [stile([128, S], BF16, f"yn{m}") for m in range(2)]      # normalized y^T
        ynodd = [stile([64, S], BF16, f"ynodd{m}") for m in range(2)]

        # ---- AllGather the weight halves across DP pairs (each weight shard
        # is sent to only one dp) and x across the DP group ----
        nc.sync.dma_start(whb[:], wh_d[:])
        nc.sync.dma_start(xgi[:], xs_d[:])
        nc.gpsimd.collective_compute(
            "AllGather", mybir.AluOpType.bypass,
            replica_groups=[[0, 4], [1, 5], [2, 6], [3, 7]],
            ins=[whb.opt()], outs=[whg.opt()])
        nc.gpsimd.collective_compute(
            "AllGather", mybir.AluOpType.bypass, replica_groups=GROUPS,
            ins=[xgi.opt()], outs=[xgo.opt()])
        # unpack gathered weight halves: half d = k-tiles 4d..4d+3 (wq, wkv)
        # and kk=d block (wo)
        for d in range(2):
            nc.sync.dma_start(wq[:, 1024 * d:1024 * (d + 1)],
                              whg[d, :, 0:1024])
            nc.sync.dma_start(wkv[:, 512 * d:512 * (d + 1)],
                              whg[d, :, 1024:1536])
            nc.sync.dma_start(wo[:, 1024 * d:1024 * (d + 1)],
                              whg[d, :, 1536:2560])
        nc.sync.dma_start(cos4_s[:], cos4_d[:])
        nc.sync.dma_start(sin4_s[:], sin4_d[:])
        nc.sync.dma_start(nsin4_s[:], nsin4_d[:])
        for p in range(2):
            nc.sync.dma_start(mask_s[p][:], mask_d[p][:])
        nc.sync.dma_start(bsel4_s[:], bsel4_d[:])
        nc.sync.dma_start(sel4_s[:], sel4_d[:])
        nc.sync.dma_start(ones64col_s[:], ones64col_d[:])
        nc.sync.dma_start(id128_s[:], id128_d[:])
        nc.sync.dma_start(qlnb_s[:], qlnb_d[:])
        nc.vector.memset(vsb[:], 1.0)  # ones column at [:, j, 64]; 0:64 overwritten below
        nc.vector.memset(epsb[:], EPS)
        nc.vector.memset(zb[:], 0.0)
        nc.vector.memset(onesq[:], 1.0)

        # gathered x -> SBUF tiles  xc[k][p, 512c:512c+512] = xT[128k+p, chunk c]
        for k in range(NK):
            for c in range(4):
                nc.sync.dma_start(xc[k][:, 512 * c:512 * (c + 1)],
                                  xgo[c, 128 * k:128 * (k + 1), :])

        # ======== phase 1: projections + rms factors + rope ========
        with (
            tc.tile_pool(name="pp", bufs=1, space=bass.MemorySpace.PSUM) as pp,
        ):
            # Q projection -> qsb (permuted: tileT = tops of 4 heads, tileB = bottoms)
            for m in range(2):
                pq = pp.tile([128, S], F32, name="pq", tag="pq")
                for k in range(NK):
                    for n in range(4):
                        nc.tensor.matmul(
                            pq[:, 512 * n:512 * (n + 1)],
                            wq[:, 256 * k + 128 * m:256 * k + 128 * (m + 1)],
                            xc[k][:, 512 * n:512 * (n + 1)],
                            start=(k == 0), stop=(k == NK - 1))
                nc.scalar.copy(qsb[m][:], pq[:])
                nc.vector.tensor_mul(sqq[m][:], qsb[m][:], qsb[m][:])
            # KV projection
            pkv = pp.tile([128, S], F32, name="pq", tag="pq")
            for k in range(NK):
                for n in range(4):
                    nc.tensor.matmul(
                        pkv[:, 512 * n:512 * (n + 1)],
                        wkv[:, 128 * k:128 * (k + 1)],
                        xc[k][:, 512 * n:512 * (n + 1)],
                        start=(k == 0), stop=(k == NK - 1))
            nc.scalar.copy(kvsb[:], pkv[:])
            nc.vector.tensor_mul(sqkv[:], kvsb[0:64, :], kvsb[0:64, :])
            # v transpose: [64,128] slices -> [128,64] columns of vsb
            for st in range(NJ):
                ptr = pp.tile([128, 64], BF16, name="ptr", tag="ptr", bufs=2)
                nc.tensor.transpose(
                    ptr[:], kvsb[64:128, 128 * st:128 * (st + 1)],
                    id128_s[64:128, 64:128])
                nc.vector.tensor_copy(vsb[:, st, 0:64], ptr[:])

            # rms factors: f = exp(-0.5*ln(ssq/HD + eps) + ln(gain/8))
            psq = pp.tile([4, S], F32, name="psq", tag="pq")
            for n in range(4):
                sl = slice(512 * n, 512 * (n + 1))
                nc.tensor.matmul(psq[0:4, sl], sel4_s[:], sqq[0][:, sl],
                                 start=True, stop=False)
                nc.tensor.matmul(psq[0:4, sl], sel4_s[:], sqq[1][:, sl],
                                 start=False, stop=True)
            nc.scalar.activation(lnq[:], psq[:], AF.Ln, scale=1.0 / HD,
                                 bias=epsb[0:4, :])
            psk = pp.tile([1, S], F32, name="psk", tag="pq")
            for n in range(4):
                sl = slice(512 * n, 512 * (n + 1))
                nc.tensor.matmul(psk[0:1, sl], ones64col_s[:], sqkv[:, sl],
                                 start=True, stop=True)
            nc.scalar.activation(lnk[:], psk[:], AF.Ln, scale=1.0 / HD,
                                 bias=epsb[0:1, :])
            nc.scalar.activation(fq[:], lnq[:], AF.Exp, scale=-0.5, bias=qlnb_s[:, :])
            nc.scalar.activation(fk[:], lnk[:], AF.Exp, scale=-0.5, bias=zb[0:1, :])
            # broadcast factors along hd rows via PE
            pbq = pp.tile([128, S], F32, name="pbq", tag="pq")
            for n in range(4):
                sl = slice(512 * n, 512 * (n + 1))
                nc.tensor.matmul(pbq[:, sl], bsel4_s[:], fq[:, sl],
                                 start=True, stop=True)
            nc.scalar.copy(fbcq[:], pbq[:])
            pbk = pp.tile([32, S], F32, name="pbk", tag="pq")
            for n in range(4):
                sl = slice(512 * n, 512 * (n + 1))
                nc.tensor.matmul(pbk[0:32, sl], onesq[0:1, 0:32], fk[:, sl],
                                 start=True, stop=True)
            nc.scalar.copy(fbck[:], pbk[:])

            # k bottom half shifted to partition base 0 (DVE ops need aligned bases)
            nc.sync.dma_start(kb0[:], kvsb[32:64, :])

            # rope + scale (DVE, bf16), in two column halves so the first
            # attention blocks start before the second half finishes
            with tc.tile_pool(name="rt", bufs=2) as rt:
                for hf in range(2):
                    sl = slice(1024 * hf, 1024 * (hf + 1))
                    t1 = rt.tile([128, 1024], BF16, name="t1", tag="t1")
                    t2 = rt.tile([128, 1024], BF16, name="t2", tag="t2")
                    nc.vector.tensor_mul(t1[:], qsb[0][:, sl], cos4_s[:, sl])
                    nc.vector.tensor_mul(t2[:], qsb[1][:, sl], sin4_s[:, sl])
                    nc.vector.tensor_add(t1[:], t1[:], t2[:])
                    nc.vector.tensor_mul(qr[0][:, sl], t1[:], fbcq[:, sl])
                    u1 = rt.tile([128, 1024], BF16, name="t1", tag="t1")
                    u2 = rt.tile([128, 1024], BF16, name="t2", tag="t2")
                    nc.vector.tensor_mul(u1[:], qsb[0][:, sl], nsin4_s[:, sl])
                    nc.vector.tensor_mul(u2[:], qsb[1][:, sl], cos4_s[:, sl])
                    nc.vector.tensor_add(u1[:], u1[:], u2[:])
                    nc.vector.tensor_mul(qr[1][:, sl], u1[:], fbcq[:, sl])
                    k1 = rt.tile([32, 1024], BF16, name="k1", tag="k1")
                    k2 = rt.tile([32, 1024], BF16, name="k2", tag="k2")
                    nc.vector.tensor_mul(k1[:], kvsb[0:32, sl], cos4_s[0:32, sl])
                    nc.vector.tensor_mul(k2[:], kb0[:, sl], sin4_s[0:32, sl])
                    nc.vector.tensor_add(k1[:], k1[:], k2[:])
                    nc.vector.tensor_mul(kr[0][:, sl], k1[:], fbck[:, sl])
                    k3 = rt.tile([32, 1024], BF16, name="k1", tag="k1")
                    k4 = rt.tile([32, 1024], BF16, name="k2", tag="k2")
                    nc.vector.tensor_mul(k3[:], kvsb[0:32, sl], nsin4_s[0:32, sl])
                    nc.vector.tensor_mul(k4[:], kb0[:, sl], cos4_s[0:32, sl])
                    nc.vector.tensor_add(k3[:], k3[:], k4[:])
                    nc.vector.tensor_mul(kr[1][:, sl], k3[:], fbck[:, sl])
                    # reassemble per-head-pair layout qAB[dim, head%2, s] for
                    # this half: head h dims = [qr[0] (rot top); qr[1] (bot)]
                    nc.sync.dma_start(kdup[0:32, sl], kr[0][:, sl])
                    nc.sync.dma_start(kdup[32:64, sl], kr[1][:, sl])
                    nc.sync.dma_start(kdup[64:96, sl], kr[0][:, sl])
                    nc.sync.dma_start(kdup[96:128, sl], kr[1][:, sl])
                    for h in range(4):
                        pb_ = 64 * (h // 2)   # pair A rows 0:64, pair B 64:128
                        fi = h % 2            # free index within the pair
                        nc.sync.dma_start(qAB[pb_:pb_ + 32, fi, sl],
                                          qr[0][32 * h:32 * h + 32, sl])
                        nc.sync.dma_start(qAB[pb_ + 32:pb_ + 64, fi, sl],
                                          qr[1][32 * h:32 * h + 32, sl])

        # ======== phase 2 + fused output projection ========
        with (
            tc.tile_pool(name="ps", bufs=2, space=bass.MemorySpace.PSUM) as ps,
            tc.tile_pool(name="py", bufs=2, space=bass.MemorySpace.PSUM) as py,
            tc.tile_pool(name="po", bufs=2, space=bass.MemorySpace.PSUM) as po,
            tc.tile_pool(name="pa", bufs=3) as pa,
            tc.tile_pool(name="ob", bufs=3) as ob,
        ):
            for b in range(NB):
                sq = slice(SQB * b, SQB * (b + 1))
                jmax = 2 * b + 1
                # yt[pair] accumulates [v|ones]^T @ p : rows 0:64 y, row 64 denom
                yt = [py.tile([65, 512], F32, name="yt", tag="yt") for _ in range(2)]
                for j in range(jmax + 1):
                    stile_ = ps.tile([128, HG * SQB], F32, name="st", tag="st")
                    for pr in range(2):
                        base = 64 * pr
                        nc.tensor.matmul(
                            stile_[:, 512 * pr:512 * (pr + 1)],
                            kdup[base:base + 64, 128 * j:128 * (j + 1)],
                            qAB[base:base + 64, :, sq],
                            start=True, stop=True, skip_group_check=True)
                    pt = pa.tile([128, HG * SQB], BF16, name="pt", tag="pt")
                    nc.scalar.activation(pt[:], stile_[:], AF.Exp, bias=zb[:, :])
                    if j >= 2 * b:
                        nc.vector.tensor_mul(pt[:], pt[:], mask_s[j - 2 * b][:])
                    if KDEBUG and b == 0:
                        nc.sync.dma_start(dbg[f"d_pt{j}"][:], pt[:])
                    for pr in range(2):
                        nc.tensor.matmul(
                            yt[pr][:], vsb[:, j, :], pt[:, 512 * pr:512 * (pr + 1)],
                            start=(j == 0), stop=(j == jmax))
                # normalize: y / denom -> yn rows (even head) + ynodd staging (odd)
                for pr in range(2):
                    dcb = pa.tile([1, 512], BF16, name="dcb", tag="dcb")
                    nc.vector.tensor_copy(dcb[:], yt[pr][64:65, :])
                    prb = ps.tile([64, 512], F32, name="prb", tag="st")
                    nc.tensor.matmul(prb[:], onesq[0:1, 0:64], dcb[:],
                                     start=True, stop=True)
                    rbs = pa.tile([64, 512], F32, name="rbs", tag="rbs")
                    nc.vector.reciprocal_approx_fast(rbs[:], prb[:])
                    nc.vector.tensor_mul(yn[pr][0:64, sq], yt[pr][0:64, 0:256],
                                         rbs[:, 0:256])
                    nc.vector.tensor_mul(ynodd[pr][:, sq], yt[pr][0:64, 256:512],
                                         rbs[:, 256:512])
                    nc.sync.dma_start(yn[pr][64:128, sq], ynodd[pr][:, sq])
                # fused output projection for this block's two row-tiles
                for st in (2 * b, 2 * b + 1):
                    ssl = slice(128 * st, 128 * (st + 1))
                    for n in range(2):
                        pot = po.tile([128, 512], F32, name="po", tag="po")
                        for kk in range(2):
                            nc.tensor.matmul(
                                pot[:], yn[kk][:, ssl],
                                wo[:, 1024 * kk + 512 * n:1024 * kk + 512 * (n + 1)],
                                start=(kk == 0), stop=(kk == 1))
                        ot = ob.tile([128, 512], BF16, name="ot", tag="ot")
                        nc.vector.tensor_copy(ot[:], pot[:])
                        nc.sync.dma_start(yb[ssl, 512 * n:512 * (n + 1)], ot[:])
        if KDEBUG:
            nc.sync.dma_start(dbg["d_qsb0"][:], qsb[0][:])
            nc.sync.dma_start(dbg["d_qsb1"][:], qsb[1][:])
            nc.sync.dma_start(dbg["d_kvsb"][:], kvsb[:])
            nc.sync.dma_start(dbg["d_fq"][:], fq[:])
            nc.sync.dma_start(dbg["d_fk"][:], fk[:])
            nc.sync.dma_start(dbg["d_qr0"][:], qr[0][:])
            nc.sync.dma_start(dbg["d_qr1"][:], qr[1][:])
            nc.sync.dma_start(dbg["d_kdup"][:], kdup[:])
            nc.sync.dma_start(dbg["d_qAB"][:, 0:S], qAB[:, 0, :])
            nc.sync.dma_start(dbg["d_qAB"][:, S:2 * S], qAB[:, 1, :])
            for jj in range(NJ):
                nc.sync.dma_start(dbg["d_vsb"][:, 65 * jj:65 * (jj + 1)],
                                  vsb[:, jj, :])
            nc.sync.dma_start(dbg["d_yn0"][:], yn[0][:])
            nc.sync.dma_start(dbg["d_yn1"][:], yn[1][:])

        # ======== ReduceScatter the partials, write the output chunk ========
        nc.gpsimd.collective_compute(
            "ReduceScatter", mybir.AluOpType.add, replica_groups=GROUPS,
            ins=[yb.opt()], outs=[rso.opt()])
        nc.sync.dma_start(out_d[:], rso[:])

    nc.finalize()
    return nc


_NC = None


def _get_nc():
    global _NC
    if _NC is None:
        _NC = _build()
    return _NC


def _perm():
    tops = [h * 64 + i for h in range(HG) for i in range(32)]
    bots = [h * 64 + 32 + i for h in range(HG) for i in range(32)]
    return tops + bots


def _build_in_maps(x, Wq, Wk, Wv, Wo, q_gain):
    perm = _perm()
    in_maps = []
    for c in range(8):
        dp, tp = divmod(c, 4)
        xs = np.ascontiguousarray(
            x[dp].T[:, tp * SC:(tp + 1) * SC]).astype(BF16NP)     # [D, 512]
        wq_sel = Wq[tp * E:(tp + 1) * E].T[:, perm]               # [D, 256] permuted
        wq_t = np.ascontiguousarray(
            wq_sel.reshape(NK, 128, E).transpose(1, 0, 2).reshape(128, NK * E)
        ).astype(BF16NP)
        wk_sel = Wk[tp * HD:(tp + 1) * HD].T                      # [D, 64]
        wv_sel = Wv[tp * HD:(tp + 1) * HD].T
        wkv_sel = np.concatenate([wk_sel, wv_sel], axis=1)        # [D, 128]
        wkv_t = np.ascontiguousarray(
            wkv_sel.reshape(NK, 128, 128).transpose(1, 0, 2).reshape(128, NK * 128)
        ).astype(BF16NP)
        wo_sel = Wo[:, tp * E:(tp + 1) * E].T                     # [256, D]
        wo_t = np.ascontiguousarray(
            wo_sel.reshape(2, 128, D).transpose(1, 0, 2).reshape(128, 2 * D)
        ).astype(BF16NP)
        # each core ships only half its weight shard; the DP-pair AllGather
        # reassembles the full shard on device
        wh = np.concatenate([wq_t[:, 1024 * dp:1024 * (dp + 1)],
                             wkv_t[:, 512 * dp:512 * (dp + 1)],
                             wo_t[:, 1024 * dp:1024 * (dp + 1)]], axis=1)
        g = q_gain[tp * HG:(tp + 1) * HG].astype(np.float64)
        qlnb = np.log(np.maximum(g, 1e-30) / 8.0).astype(np.float32).reshape(4, 1)
        in_maps.append({
            "xs": xs, "wh": np.ascontiguousarray(wh), "qlnb": qlnb,
        })
    return in_maps


def kernel(x, Wq, Wk, Wv, Wo, q_gain):
    x = np.asarray(x, dtype=np.float32)
    Wq = np.asarray(Wq, dtype=np.float32)
    Wk = np.asarray(Wk, dtype=np.float32)
    Wv = np.asarray(Wv, dtype=np.float32)
    Wo = np.asarray(Wo, dtype=np.float32)
    q_gain = np.asarray(q_gain, dtype=np.float32)

    in_maps = _build_in_maps(x, Wq, Wk, Wv, Wo, q_gain)
    nc = _get_nc()
    res = run_bass_kernel_spmd(nc, in_maps, core_ids=list(range(8)))
    out = np.zeros((B, S, D), dtype=np.float32)
    for c in range(8):
        dp, tp = divmod(c, 4)
        out[dp, tp * SC:(tp + 1) * SC, :] = res.results[c]["out"].astype(np.float32)
    return out


# revision 44
# speedup vs baseline: 1.0507x; 1.0120x over previous
"""Trainium2 Bass kernel for causal GQA self-attention (B=2,S=2048,D=1024,H=16,HKV=4,HD=64).

Sharding: 8 cores = DP(2 over batch) x TP(4 over GQA groups).
I/O-lean layout:
  - x arrives seq-sharded: core (dp,tp) receives xT[:, tp*512:(tp+1)*512] of batch dp
    (1MB bf16); an on-device AllGather over the DP group rebuilds the full xT.
  - each core computes its GQA group's attention partial y_group @ Wo[:, group].T,
    then an on-device ReduceScatter sums the 4 TP partials and hands core tp
    rows [tp*512:(tp+1)*512]; the core returns that chunk in bf16 (1MB).
Host just concatenates chunks - no reduction on host.
"""

import sys
from contextlib import ExitStack

sys.path.insert(0, "/opt/trn_rl_repo")

import numpy as np
import ml_dtypes

import concourse.bass as bass
import concourse.bacc as bacc
import concourse.tile as tile
import concourse.mybir as mybir
from concourse.bass_utils import run_bass_kernel_spmd

BF16 = mybir.dt.bfloat16
F32 = mybir.dt.float32
AF = mybir.ActivationFunctionType
BF16NP = ml_dtypes.bfloat16

D, H, HKV, HD, B, S = 1024, 16, 4, 64, 2, 2048
HG = 4              # q heads per core
E = HG * HD         # 256 local q-proj dim
ROPE_BASE = 10000.0
EPS = float(np.finfo(np.float32).eps)

NK = D // 128       # 8 contraction tiles for qkv projections
SQB = 256           # sq block size in attention
NB = S // SQB       # 8 blocks
NJ = S // 128       # 16 sk tiles
SC = S // 4         # 512 seq chunk per core
GROUPS = [[0, 1, 2, 3], [4, 5, 6, 7]]


def _consts():
    """Constant tensors baked into the NEFF (same for every core)."""
    i = np.arange(32, dtype=np.float64)
    inv_freq = 1.0 / (ROPE_BASE ** (2.0 * i / HD))
    pos = np.arange(S, dtype=np.float64)
    fr = pos[:, None] * inv_freq[None, :]          # [S, 32]
    cosT = np.cos(fr).T.astype(np.float32)          # [32, S]
    sinT = np.sin(fr).T.astype(np.float32)
    cos4 = np.tile(cosT, (4, 1)).astype(BF16NP)     # [128, S]
    sin4 = np.tile(sinT, (4, 1)).astype(BF16NP)
    nsin4 = (-np.tile(sinT, (4, 1))).astype(BF16NP)

    # causal masks for diagonal sk-tiles: pattern p in {0,1}
    # valid iff c >= 128*p + r   (r: sk row 0..127, c: sq col 0..255)
    r = np.arange(128)[:, None]
    c = np.arange(SQB)[None, :]
    masks = []
    for p in range(2):
        m = (c >= 128 * p + r).astype(BF16NP)       # [128, 256]
        masks.append(np.tile(m, (1, HG)))            # [128, 1024] (4 head blocks)

    bsel4 = np.zeros((4, 128), dtype=BF16NP)        # broadcast f[h] -> rows 32h..32h+32
    for h in range(4):
        bsel4[h, 32 * h:32 * h + 32] = 1.0
    sel4 = bsel4.T.copy()                            # [128, 4] sumsq selector
    ones64col = np.ones((64, 1), dtype=BF16NP)
    id128 = np.eye(128, dtype=BF16NP)
    return cos4, sin4, nsin4, masks, bsel4, sel4, ones64col, id128


def _build():
    nc = bacc.Bacc("TRN2", debug=False)

    xs_d = nc.dram_tensor("xs", [D, SC], BF16, kind="ExternalInput")
    # packed half of this TP shard's weights: [wq k0-3 | wkv k0-3 | wo kk] halves
    WHW = (NK * E + NK * 128 + 2 * D) // 2          # 2560
    wh_d = nc.dram_tensor("wh", [128, WHW], BF16, kind="ExternalInput")
    qlnb_d = nc.dram_tensor("qlnb", [4, 1], F32, kind="ExternalInput")
    out_d = nc.dram_tensor("out", [SC, D], BF16, kind="ExternalOutput")
    import os
    KDEBUG = int(os.environ.get("KDEBUG", "0"))
    dbg = {}
    if KDEBUG:
        for nm, shp in [("d_qsb0", [128, S]), ("d_qsb1", [128, S]),
                        ("d_kvsb", [128, S]), ("d_fq", [4, S]), ("d_fk", [1, S]),
                        ("d_qr0", [128, S]), ("d_qr1", [128, S]),
                        ("d_kdup", [128, S]), ("d_qAB", [128, 2 * S]),
                        ("d_vsb", [128, NJ * 65]),
                        ("d_pt0", [128, HG * SQB]), ("d_pt1", [128, HG * SQB]),
                        ("d_yn0", [128, S]), ("d_yn1", [128, S])]:
            dbg[nm] = nc.dram_tensor(nm, shp, BF16, kind="ExternalOutput")

    cos4, sin4, nsin4, masks, bsel4, sel4, ones64col, id128 = _consts()
    cos4_d = nc.inline_tensor(cos4, "cos4")
    sin4_d = nc.inline_tensor(sin4, "sin4")
    nsin4_d = nc.inline_tensor(nsin4, "nsin4")
    mask_d = [nc.inline_tensor(masks[p], f"mask{p}") for p in range(2)]
    bsel4_d = nc.inline_tensor(bsel4, "bsel4")
    sel4_d = nc.inline_tensor(sel4, "sel4")
    ones64col_d = nc.inline_tensor(ones64col, "ones64col")
    id128_d = nc.inline_tensor(id128, "id128")

    with tile.TileContext(nc) as tc, ExitStack() as ctx:
        sp = ctx.enter_context(tc.tile_pool(name="static", bufs=1))
        dram = ctx.enter_context(tc.tile_pool(name="dram", bufs=1, space="DRAM"))

        def stile(shape, dt, tag):
            return sp.tile(shape, dt, name=tag, tag=tag)

        # ---- DRAM internals for collectives ----
        xgi = dram.tile([D, SC], BF16, name="xgi", tag="xgi")
        xgo = dram.tile([4, D, SC], BF16, name="xgo", tag="xgo")
        whb = dram.tile([128, WHW], BF16, name="whb", tag="whb")
        whg = dram.tile([2, 128, WHW], BF16, name="whg", tag="whg")
        yb = dram.tile([S, D], BF16, name="yb", tag="yb")
        rso = dram.tile([SC, D], BF16, name="rso", tag="rso")

        # ---- static SBUF tensors ----
        xc = [stile([128, S], BF16, f"xc{k}") for k in range(NK)]
        wq = stile([128, NK * E], BF16, "wq")
        wkv = stile([128, NK * 128], BF16, "wkv")
        wo = stile([128, 2 * D], BF16, "wo")
        cos4_s = stile([128, S], BF16, "cos4")
        sin4_s = stile([128, S], BF16, "sin4")
        nsin4_s = stile([128, S], BF16, "nsin4")
        mask_s = [stile([128, HG * SQB], BF16, f"mask{p}") for p in range(2)]
        bsel4_s = stile([4, 128], BF16, "bsel4")
        sel4_s = stile([128, 4], BF16, "sel4")
        ones64col_s = stile([64, 1], BF16, "ones64col")
        id128_s = stile([128, 128], BF16, "id128")
        qlnb_s = stile([4, 1], F32, "qlnb")
        epsb = stile([128, 1], F32, "epsb")
        zb = stile([128, 1], F32, "zb")

        qsb = [stile([128, S], BF16, f"qsb{m}") for m in range(2)]   # T/B packed
        kvsb = stile([128, S], BF16, "kvsb")                          # k(0:64) | v(64:128)
        sqq = [stile([128, S], BF16, f"sqq{m}") for m in range(2)]
        sqkv = stile([64, S], BF16, "sqkv")
        lnq = stile([4, S], F32, "lnq")
        lnk = stile([1, S], F32, "lnk")
        fq = stile([4, S], BF16, "fq")
        fk = stile([1, S], BF16, "fk")
        fbcq = stile([128, S], BF16, "fbcq")
        fbck = stile([32, S], BF16, "fbck")
        qr = [stile([128, S], BF16, f"qr{m}") for m in range(2)]      # rotated T/B
        kr = [stile([32, S], BF16, f"kr{m}") for m in range(2)]
        kb0 = stile([32, S], BF16, "kb0")
        qAB = stile([128, 2, S], BF16, "qAB")  # rows 0:64 heads 0|1, 64:128 heads 2|3
        kdup = stile([128, S], BF16, "kdup")
        onesq = stile([128, 64], BF16, "onesq")
        vsb = stile([128, NJ, 65], BF16, "vsb")                       # [v | ones]
        yn = [stile([128, S], BF16, f"yn{m}") for m in range(2)]      # normalized y^T
        ynodd = [stile([64, S], BF16, f"ynodd{m}") for m in range(2)]

        # ---- AllGather the weight halves across DP pairs (each weight shard
        # is sent to only one dp) and x across the DP group ----
        nc.sync.dma_start(whb[:], wh_d[:])
        nc.sync.dma_start(xgi[:], xs_d[:])
        nc.gpsimd.collective_compute(
            "AllGather", mybir.AluOpType.bypass,
            replica_groups=[[0, 4], [1, 5], [2, 6], [3, 7]],
            ins=[whb.opt()], outs=[whg.opt()])
        nc.gpsimd.collective_compute(
            "AllGather", mybir.AluOpType.bypass, replica_groups=GROUPS,
            ins=[xgi.opt()], outs=[xgo.opt()])
        # unpack gathered weight halves: half d = k-tiles 4d..4d+3 (wq, wkv)
        # and kk=d block (wo)
        for d in range(2):
            nc.sync.dma_start(wq[:, 1024 * d:1024 * (d + 1)],
                              whg[d, :, 0:1024])
            nc.sync.dma_start(wkv[:, 512 * d:512 * (d + 1)],
                              whg[d, :, 1024:1536])
            nc.sync.dma_start(wo[:, 1024 * d:1024 * (d + 1)],
                              whg[d, :, 1536:2560])
        nc.sync.dma_start(cos4_s[:], cos4_d[:])
        nc.sync.dma_start(sin4_s[:], sin4_d[:])
        nc.sync.dma_start(nsin4_s[:], nsin4_d[:])
        for p in range(2):
            nc.sync.dma_start(mask_s[p][:], mask_d[p][:])
        nc.sync.dma_start(bsel4_s[:], bsel4_d[:])
        nc.sync.dma_start(sel4_s[:], sel4_d[:])
        nc.sync.dma_start(ones64col_s[:], ones64col_d[:])
        nc.sync.dma_start(id128_s[:], id128_d[:])
        nc.sync.dma_start(qlnb_s[:], qlnb_d[:])
        nc.vector.memset(vsb[:], 1.0)  # ones column at [:, j, 64]; 0:64 overwritten below
        nc.vector.memset(epsb[:], EPS)
        nc.vector.memset(zb[:], 0.0)
        nc.vector.memset(onesq[:], 1.0)

        # gathered x -> SBUF tiles  xc[k][p, 512c:512c+512] = xT[128k+p, chunk c]
        # (alternate DMA queues: halves the serialized dispatch on one ring)
        for k in range(NK):
            for c in range(4):
                q = nc.sync if (k * 4 + c) % 2 == 0 else nc.gpsimd
                q.dma_start(xc[k][:, 512 * c:512 * (c + 1)],
                            xgo[c, 128 * k:128 * (k + 1), :])

        # ======== phase 1: projections + rms factors + rope ========
        with (
            tc.tile_pool(name="pp", bufs=1, space=bass.MemorySpace.PSUM) as pp,
        ):
            # Q projection -> qsb (permuted: tileT = tops of 4 heads, tileB = bottoms)
            for m in range(2):
                pq = pp.tile([128, S], F32, name="pq", tag="pq")
                for k in range(NK):
                    for n in range(4):
                        nc.tensor.matmul(
                            pq[:, 512 * n:512 * (n + 1)],
                            wq[:, 256 * k + 128 * m:256 * k + 128 * (m + 1)],
                            xc[k][:, 512 * n:512 * (n + 1)],
                            start=(k == 0), stop=(k == NK - 1))
                nc.scalar.copy(qsb[m][:], pq[:])
                nc.vector.tensor_mul(sqq[m][:], qsb[m][:], qsb[m][:])
            # KV projection
            pkv = pp.tile([128, S], F32, name="pq", tag="pq")
            for k in range(NK):
                for n in range(4):
                    nc.tensor.matmul(
                        pkv[:, 512 * n:512 * (n + 1)],
                        wkv[:, 128 * k:128 * (k + 1)],
                        xc[k][:, 512 * n:512 * (n + 1)],
                        start=(k == 0), stop=(k == NK - 1))
            nc.scalar.copy(kvsb[:], pkv[:])
            nc.vector.tensor_mul(sqkv[:], kvsb[0:64, :], kvsb[0:64, :])
            # v transpose: [64,128] slices -> [128,64] columns of vsb
            for st in range(NJ):
                ptr = pp.tile([128, 64], BF16, name="ptr", tag="ptr", bufs=2)
                nc.tensor.transpose(
                    ptr[:], kvsb[64:128, 128 * st:128 * (st + 1)],
                    id128_s[64:128, 64:128])
                nc.vector.tensor_copy(vsb[:, st, 0:64], ptr[:])

            # rms factors: f = exp(-0.5*ln(ssq/HD + eps) + ln(gain/8))
            psq = pp.tile([4, S], F32, name="psq", tag="pq")
            for n in range(4):
                sl = slice(512 * n, 512 * (n + 1))
                nc.tensor.matmul(psq[0:4, sl], sel4_s[:], sqq[0][:, sl],
                                 start=True, stop=False)
                nc.tensor.matmul(psq[0:4, sl], sel4_s[:], sqq[1][:, sl],
                                 start=False, stop=True)
            nc.scalar.activation(lnq[:], psq[:], AF.Ln, scale=1.0 / HD,
                                 bias=epsb[0:4, :])
            psk = pp.tile([1, S], F32, name="psk", tag="pq")
            for n in range(4):
                sl = slice(512 * n, 512 * (n + 1))
                nc.tensor.matmul(psk[0:1, sl], ones64col_s[:], sqkv[:, sl],
                                 start=True, stop=True)
            nc.scalar.activation(lnk[:], psk[:], AF.Ln, scale=1.0 / HD,
                                 bias=epsb[0:1, :])
            nc.scalar.activation(fq[:], lnq[:], AF.Exp, scale=-0.5, bias=qlnb_s[:, :])
            nc.scalar.activation(fk[:], lnk[:], AF.Exp, scale=-0.5, bias=zb[0:1, :])
            # broadcast factors along hd rows via PE
            pbq = pp.tile([128, S], F32, name="pbq", tag="pq")
            for n in range(4):
                sl = slice(512 * n, 512 * (n + 1))
                nc.tensor.matmul(pbq[:, sl], bsel4_s[:], fq[:, sl],
                                 start=True, stop=True)
            nc.scalar.copy(fbcq[:], pbq[:])
            pbk = pp.tile([32, S], F32, name="pbk", tag="pq")
            for n in range(4):
                sl = slice(512 * n, 512 * (n + 1))
                nc.tensor.matmul(pbk[0:32, sl], onesq[0:1, 0:32], fk[:, sl],
                                 start=True, stop=True)
            nc.scalar.copy(fbck[:], pbk[:])

            # k bottom half shifted to partition base 0 (DVE ops need aligned bases)
            nc.sync.dma_start(kb0[:], kvsb[32:64, :])

            # rope + scale (DVE, bf16), in two column halves so the first
            # attention blocks start before the second half finishes
            with tc.tile_pool(name="rt", bufs=2) as rt:
                for hf in range(2):
                    sl = slice(1024 * hf, 1024 * (hf + 1))
                    t1 = rt.tile([128, 1024], BF16, name="t1", tag="t1")
                    t2 = rt.tile([128, 1024], BF16, name="t2", tag="t2")
                    nc.vector.tensor_mul(t1[:], qsb[0][:, sl], cos4_s[:, sl])
                    nc.vector.tensor_mul(t2[:], qsb[1][:, sl], sin4_s[:, sl])
                    nc.vector.tensor_add(t1[:], t1[:], t2[:])
                    nc.vector.tensor_mul(qr[0][:, sl], t1[:], fbcq[:, sl])
                    u1 = rt.tile([128, 1024], BF16, name="t1", tag="t1")
                    u2 = rt.tile([128, 1024], BF16, name="t2", tag="t2")
                    nc.vector.tensor_mul(u1[:], qsb[0][:, sl], nsin4_s[:, sl])
                    nc.vector.tensor_mul(u2[:], qsb[1][:, sl], cos4_s[:, sl])
                    nc.vector.tensor_add(u1[:], u1[:], u2[:])
                    nc.vector.tensor_mul(qr[1][:, sl], u1[:], fbcq[:, sl])
                    k1 = rt.tile([32, 1024], BF16, name="k1", tag="k1")
                    k2 = rt.tile([32, 1024], BF16, name="k2", tag="k2")
                    nc.vector.tensor_mul(k1[:], kvsb[0:32, sl], cos4_s[0:32, sl])
                    nc.vector.tensor_mul(k2[:], kb0[:, sl], sin4_s[0:32, sl])
                    nc.vector.tensor_add(k1[:], k1[:], k2[:])
                    nc.vector.tensor_mul(kr[0][:, sl], k1[:], fbck[:, sl])
                    k3 = rt.tile([32, 1024], BF16, name="k1", tag="k1")
                    k4 = rt.tile([32, 1024], BF16, name="k2", tag="k2")
                    nc.vector.tensor_mul(k3[:], kvsb[0:32, sl], nsin4_s[0:32, sl])
                    nc.vector.tensor_mul(k4[:], kb0[:, sl], cos4_s[0:32, sl])
                    nc.vector.tensor_add(k3[:], k3[:], k4[:])
                    nc.vector.tensor_mul(kr[1][:, sl], k3[:], fbck[:, sl])
                    # reassemble per-head-pair layout qAB[dim, head%2, s] for
                    # this half: head h dims = [qr[0] (rot top); qr[1] (bot)]
                    nc.sync.dma_start(kdup[0:32, sl], kr[0][:, sl])
                    nc.gpsimd.dma_start(kdup[32:64, sl], kr[1][:, sl])
                    nc.sync.dma_start(kdup[64:96, sl], kr[0][:, sl])
                    nc.gpsimd.dma_start(kdup[96:128, sl], kr[1][:, sl])
                    for h in range(4):
                        pb_ = 64 * (h // 2)   # pair A rows 0:64, pair B 64:128
                        fi = h % 2            # free index within the pair
                        nc.sync.dma_start(qAB[pb_:pb_ + 32, fi, sl],
                                          qr[0][32 * h:32 * h + 32, sl])
                        nc.gpsimd.dma_start(qAB[pb_ + 32:pb_ + 64, fi, sl],
                                            qr[1][32 * h:32 * h + 32, sl])

        # ======== phase 2 + fused output projection ========
        with (
            tc.tile_pool(name="ps", bufs=2, space=bass.MemorySpace.PSUM) as ps,
            tc.tile_pool(name="py", bufs=2, space=bass.MemorySpace.PSUM) as py,
            tc.tile_pool(name="po", bufs=2, space=bass.MemorySpace.PSUM) as po,
            tc.tile_pool(name="pa", bufs=3) as pa,
            tc.tile_pool(name="ob", bufs=3) as ob,
        ):
            for b in range(NB):
                sq = slice(SQB * b, SQB * (b + 1))
                jmax = 2 * b + 1
                # yt[pair] accumulates [v|ones]^T @ p : rows 0:64 y, row 64 denom
                yt = [py.tile([65, 512], F32, name="yt", tag="yt") for _ in range(2)]
                for j in range(jmax + 1):
                    stile_ = ps.tile([128, HG * SQB], F32, name="st", tag="st")
                    for pr in range(2):
                        base = 64 * pr
                        nc.tensor.matmul(
                            stile_[:, 512 * pr:512 * (pr + 1)],
                            kdup[base:base + 64, 128 * j:128 * (j + 1)],
                            qAB[base:base + 64, :, sq],
                            start=True, stop=True, skip_group_check=True)
                    pt = pa.tile([128, HG * SQB], BF16, name="pt", tag="pt")
                    nc.scalar.activation(pt[:], stile_[:], AF.Exp, bias=zb[:, :])
                    if j >= 2 * b:
                        nc.vector.tensor_mul(pt[:], pt[:], mask_s[j - 2 * b][:])
                    if KDEBUG and b == 0:
                        nc.sync.dma_start(dbg[f"d_pt{j}"][:], pt[:])
                    for pr in range(2):
                        nc.tensor.matmul(
                            yt[pr][:], vsb[:, j, :], pt[:, 512 * pr:512 * (pr + 1)],
                            start=(j == 0), stop=(j == jmax))
                # normalize: y / denom -> yn rows (even head) + ynodd staging (odd)
                for pr in range(2):
                    dcb = pa.tile([1, 512], BF16, name="dcb", tag="dcb")
                    nc.vector.tensor_copy(dcb[:], yt[pr][64:65, :])
                    prb = ps.tile([64, 512], F32, name="prb", tag="st")
                    nc.tensor.matmul(prb[:], onesq[0:1, 0:64], dcb[:],
                                     start=True, stop=True)
                    rbs = pa.tile([64, 512], F32, name="rbs", tag="rbs")
                    nc.vector.reciprocal_approx_fast(rbs[:], prb[:])
                    nc.vector.tensor_mul(yn[pr][0:64, sq], yt[pr][0:64, 0:256],
                                         rbs[:, 0:256])
                    nc.vector.tensor_mul(ynodd[pr][:, sq], yt[pr][0:64, 256:512],
                                         rbs[:, 256:512])
                    nc.sync.dma_start(yn[pr][64:128, sq], ynodd[pr][:, sq])
                # fused output projection for this block's two row-tiles
                for st in (2 * b, 2 * b + 1):
                    ssl = slice(128 * st, 128 * (st + 1))
                    for n in range(2):
                        pot = po.tile([128, 512], F32, name="po", tag="po")
                        for kk in range(2):
                            nc.tensor.matmul(
                                pot[:], yn[kk][:, ssl],
                                wo[:, 1024 * kk + 512 * n:1024 * kk + 512 * (n + 1)],
                                start=(kk == 0), stop=(kk == 1))
                        ot = ob.tile([128, 512], BF16, name="ot", tag="ot")
                        nc.vector.tensor_copy(ot[:], pot[:])
                        q = nc.sync if n == 0 else nc.gpsimd
                        q.dma_start(yb[ssl, 512 * n:512 * (n + 1)], ot[:])
        if KDEBUG:
            nc.sync.dma_start(dbg["d_qsb0"][:], qsb[0][:])
            nc.sync.dma_start(dbg["d_qsb1"][:], qsb[1][:])
            nc.sync.dma_start(dbg["d_kvsb"][:], kvsb[:])
            nc.sync.dma_start(dbg["d_fq"][:], fq[:])
            nc.sync.dma_start(dbg["d_fk"][:], fk[:])
            nc.sync.dma_start(dbg["d_qr0"][:], qr[0][:])
            nc.sync.dma_start(dbg["d_qr1"][:], qr[1][:])
            nc.sync.dma_start(dbg["d_kdup"][:], kdup[:])
            nc.sync.dma_start(dbg["d_qAB"][:, 0:S], qAB[:, 0, :])
            nc.sync.dma_start(dbg["d_qAB"][:, S:2 * S], qAB[:, 1, :])
            for jj in range(NJ):
                nc.sync.dma_start(dbg["d_vsb"][:, 65 * jj:65 * (jj + 1)],
                                  vsb[:, jj, :])
            nc.sync.dma_start(dbg["d_yn0"][:], yn[0][:])
            nc.sync.dma_start(dbg["d_yn1"][:], yn[1][:])

        # ======== ReduceScatter the partials, write the output chunk ========
        nc.gpsimd.collective_compute(
            "ReduceScatter", mybir.AluOpType.add, replica_groups=GROUPS,
            ins=[yb.opt()], outs=[rso.opt()])
        nc.sync.dma_start(out_d[:], rso[:])

    nc.finalize()
    return nc


_NC = None


def _get_nc():
    global _NC
    if _NC is None:
        _NC = _build()
    return _NC


def _perm():
    tops = [h * 64 + i for h in range(HG) for i in range(32)]
    bots = [h * 64 + 32 + i for h in range(HG) for i in range(32)]
    return tops + bots


def _build_in_maps(x, Wq, Wk, Wv, Wo, q_gain):
    perm = _perm()
    in_maps = []
    for c in range(8):
        dp, tp = divmod(c, 4)
        xs = np.ascontiguousarray(
            x[dp].T[:, tp * SC:(tp + 1) * SC]).astype(BF16NP)     # [D, 512]
        wq_sel = Wq[tp * E:(tp + 1) * E].T[:, perm]               # [D, 256] permuted
        wq_t = np.ascontiguousarray(
            wq_sel.reshape(NK, 128, E).transpose(1, 0, 2).reshape(128, NK * E)
        ).astype(BF16NP)
        wk_sel = Wk[tp * HD:(tp + 1) * HD].T                      # [D, 64]
        wv_sel = Wv[tp * HD:(tp + 1) * HD].T
        wkv_sel = np.concatenate([wk_sel, wv_sel], axis=1)        # [D, 128]
        wkv_t = np.ascontiguousarray(
            wkv_sel.reshape(NK, 128, 128).transpose(1, 0, 2).reshape(128, NK * 128)
        ).astype(BF16NP)
        wo_sel = Wo[:, tp * E:(tp + 1) * E].T                     # [256, D]
        wo_t = np.ascontiguousarray(
            wo_sel.reshape(2, 128, D).transpose(1, 0, 2).reshape(128, 2 * D)
        ).astype(BF16NP)
        # each core ships only half its weight shard; the DP-pair AllGather
        # reassembles the full shard on device
        wh = np.concatenate([wq_t[:, 1024 * dp:1024 * (dp + 1)],
                             wkv_t[:, 512 * dp:512 * (dp + 1)],
                             wo_t[:, 1024 * dp:1024 * (dp + 1)]], axis=1)
        g = q_gain[tp * HG:(tp + 1) * HG].astype(np.float64)
        qlnb = np.log(np.maximum(g, 1e-30) / 8.0).astype(np.float32).reshape(4, 1)
        in_maps.append({
            "xs": xs, "wh": np.ascontiguousarray(wh), "qlnb": qlnb,
        })
    return in_maps


def kernel(x, Wq, Wk, Wv, Wo, q_gain):
    x = np.asarray(x, dtype=np.float32)
    Wq = np.asarray(Wq, dtype=np.float32)
    Wk = np.asarray(Wk, dtype=np.float32)
    Wv = np.asarray(Wv, dtype=np.float32)
    Wo = np.asarray(Wo, dtype=np.float32)
    q_gain = np.asarray(q_gain, dtype=np.float32)

    in_maps = _build_in_maps(x, Wq, Wk, Wv, Wo, q_gain)
    nc = _get_nc()
    res = run_bass_kernel_spmd(nc, in_maps, core_ids=list(range(8)))
    out = np.zeros((B, S, D), dtype=np.float32)
    for c in range(8):
        dp, tp = divmod(c, 4)
        out[dp, tp * SC:(tp + 1) * SC, :] = res.results[c]["out"].astype(np.float32)
    return out
